# revision 8
# baseline (speedup 1.0000x reference)
"""Trainium2 Bass kernel for causal self-attention with LoRA on q/v.

Reference shapes: hidden_states [4, 2048, 1024], 16 heads x 64 dims,
LoRA rank 8 (scale 2.0) on q and v projections.

Sharding: 8 cores = 4 batches x 2 head-groups. Core c handles batch
c//2 and heads (c%2)*8 .. (c%2)*8+8. Outputs are disjoint; assembled
host-side (no device collectives).

Key design points (all bf16 matmuls, fp32 PSUM accumulation):
  - LoRA is folded into the weights host-side: W' = W + 2*B@A, so the
    device sees plain q/k/v projections.
  - PE row tiling: attention matmuls have contraction dh=64, so two
    K=64 matmuls run concurrently on PE quadrants (tile_position (0,0)
    and (64,0)) for ~1.7x tensor throughput:
      * scores: the two heads of a pair (dh rows 0:64 / 64:128 of the
        projected q/k tiles) are computed concurrently.
      * PV: the s-block contraction (K=128) is split into two K=64
        halves accumulated in separate PSUM banks, summed by the DVE
        at evacuation.
  - Scores are transposed ([s-block, t] with s on partitions); exp runs
    on the scalar engine with the attention mask as per-partition bias;
    the causal diagonal block is masked post-exp by an upper-triangular
    multiply on gpsimd.
  - PV stationary is [s, 65] per (s-block, head): v columns 0:64 plus a
    ones column that accumulates the softmax denominator in PSUM row 64.
    The exp tiles are the moving operand ([65, t-chunk] output, c-major
    over four 512-col t-chunks). Unnormalized numerator + denominator
    are DMA'd out; the host does the division and transpose.
  - Projection pieces are interleaved as fillers between score blocks to
    cover the scalar engine's exp latency; PV chunks of pair p run
    c-descending, interleaved with the (descending-sb) scores of pair
    p+1 so the tensor engine never idles at pair boundaries.
"""

import sys

if "/opt/trn_rl_repo" not in sys.path:
    sys.path.insert(0, "/opt/trn_rl_repo")

import numpy as np
import ml_dtypes

BF16 = ml_dtypes.bfloat16

B, T, H, NH, DH = 4, 2048, 1024, 16, 64
N_CORES = 8
HPC = 8          # heads per core
CH = HPC * DH    # 512 output channels per core
LORA_SCALE = 2.0

_cached = {}


def _build_nc():
    import concourse.bass as bass
    import concourse.mybir as mybir
    from concourse import bacc
    from concourse.tile import TileContext

    dt = mybir.dt
    AF = mybir.ActivationFunctionType

    nc = bacc.Bacc()

    xT_d = nc.dram_tensor("xT", [4, 128, 8, 512], dt.bfloat16, kind="ExternalInput")
    wqT_d = nc.dram_tensor("wqT", [128, 8, 512], dt.bfloat16, kind="ExternalInput")
    wkT_d = nc.dram_tensor("wkT", [128, 8, 512], dt.bfloat16, kind="ExternalInput")
    wvT_d = nc.dram_tensor("wvT", [128, 8, 512], dt.bfloat16, kind="ExternalInput")
    bqk_d = nc.dram_tensor("bqk", [128, 2, 4], dt.float32, kind="ExternalInput")
    vbias_d = nc.dram_tensor("vbias", [128, 8, 64], dt.bfloat16, kind="ExternalInput")
    amask_d = nc.dram_tensor("amask", [128, 16], dt.float32, kind="ExternalInput")
    tri_d = nc.dram_tensor("tri", [128, 128], dt.bfloat16, kind="ExternalInput")
    out_d = nc.dram_tensor("out", [8, 4, 65, 512], dt.float32, kind="ExternalOutput")

    with TileContext(nc) as tc:
        with (
            tc.tile_pool(name="const", bufs=1) as cpool,
            tc.tile_pool(name="big", bufs=1) as bpool,
            tc.tile_pool(name="stage", bufs=4) as stpool,
            tc.tile_pool(name="psproj", bufs=2, space="PSUM") as ps_proj,
            tc.tile_pool(name="pssc", bufs=1, space="PSUM") as ps_sc,
            tc.tile_pool(name="pspv", bufs=1, space="PSUM") as ps_pv,
        ):
            # ---- constants ---------------------------------------------
            amask_sb = cpool.tile([128, 16], dt.float32, tag="amask")
            nc.sync.dma_start(amask_sb[:], amask_d[:])
            tri_sb = cpool.tile([128, 128], dt.bfloat16, tag="tri")
            nc.sync.dma_start(tri_sb[:], tri_d[:])
            bqk_sb = cpool.tile([128, 2, 4], dt.float32, tag="bqk")
            nc.sync.dma_start(bqk_sb[:], bqk_d[:])
            vbias_sb = cpool.tile([128, 8, 64], dt.bfloat16, tag="vbias")
            nc.sync.dma_start(vbias_sb[:], vbias_d[:])

            # ---- big persistent tensors --------------------------------
            x_sb = bpool.tile([128, 8, 2048], dt.bfloat16, tag="x")
            wq_sb = bpool.tile([128, 8, 512], dt.bfloat16, tag="wq")
            wk_sb = bpool.tile([128, 8, 512], dt.bfloat16, tag="wk")
            wv_sb = bpool.tile([128, 8, 512], dt.bfloat16, tag="wv")
            qt = [
                bpool.tile([128, 2048], dt.bfloat16, tag=f"q{j}", name=f"qt{j}")
                for j in range(4)
            ]
            kt = [
                bpool.tile([128, 2048], dt.bfloat16, tag=f"k{j}", name=f"kt{j}")
                for j in range(4)
            ]
            v_sb = bpool.tile([128, 16, 8, 65], dt.bfloat16, tag="v")
            nc.gpsimd.memset(v_sb[:, :, :, 64:65], 1.0)

            # exp tiles per (head parity, s-block)
            et = {}
            for par in range(2):
                for sb in range(16):
                    et[par, sb] = bpool.tile(
                        [128, 2048 - 128 * sb], dt.bfloat16, tag=f"e{par}_{sb}",
                        name=f"et{par}_{sb}",
                    )

            # x DMA per t-block, tb=3 first (descending-sb scores need it)
            for tb in (3, 2, 1, 0):
                nc.sync.dma_start(x_sb[:, :, tb * 512:(tb + 1) * 512], xT_d[tb])
            # j=0 slices of wq/wk first so projection can start early
            nc.sync.dma_start(wq_sb[:, :, 0:128], wqT_d[:, :, 0:128])
            nc.sync.dma_start(wk_sb[:, :, 0:128], wkT_d[:, :, 0:128])
            nc.sync.dma_start(wv_sb[:], wvT_d[:])
            nc.sync.dma_start(wq_sb[:, :, 128:512], wqT_d[:, :, 128:512])
            nc.sync.dma_start(wk_sb[:, :, 128:512], wkT_d[:, :, 128:512])

            # ---- projection pieces -------------------------------------
            def proj_qk(which, j, tb):
                w_sb, bi, dst = (wq_sb, 0, qt) if which == "q" else (wk_sb, 1, kt)
                p = ps_proj.tile([128, 512], dt.float32, tag="proj", name="pqk")
                for kc in range(8):
                    nc.tensor.matmul(
                        p[:],
                        w_sb[:, kc, j * 128:(j + 1) * 128],
                        x_sb[:, kc, tb * 512:(tb + 1) * 512],
                        start=(kc == 0),
                        stop=(kc == 7),
                    )
                nc.vector.tensor_scalar_add(
                    dst[j][:, tb * 512:(tb + 1) * 512], p[:], bqk_sb[:, bi, j:j + 1]
                )

            def proj_v(m):
                p = ps_proj.tile([128, 512], dt.float32, tag="proj", name="pv")
                for kc in range(8):
                    nc.tensor.matmul(
                        p[:],
                        x_sb[:, kc, m * 128:(m + 1) * 128],
                        wv_sb[:, kc, :],
                        start=(kc == 0),
                        stop=(kc == 7),
                    )
                nc.vector.tensor_tensor(
                    v_sb[:, m, :, 0:64],
                    p[:].rearrange("p (h d) -> p h d", h=8),
                    vbias_sb[:],
                    mybir.AluOpType.add,
                )

            # ---- scores for head pair j, s-block sb (both heads) -------
            def scores(j, sb, gap_fill=None):
                etA, etB = et[0, sb], et[1, sb]
                lhsA = kt[j][0:64, sb * 128:(sb + 1) * 128]
                lhsB = kt[j][64:128, sb * 128:(sb + 1) * 128]
                diag_c = (sb * 128) // 512
                first_ht = True
                for ht in range(2):
                    c_lo = max(2 * ht, diag_c)
                    c_hi = 2 * ht + 2
                    if c_lo >= c_hi:
                        continue
                    if not first_ht and gap_fill is not None:
                        gap_fill()
                    first_ht = False
                    scA = ps_sc.tile([128, 1024], dt.float32, tag="scA", name="scA")
                    scB = ps_sc.tile([128, 1024], dt.float32, tag="scB", name="scB")
                    for c in range(c_lo, c_hi):
                        r = sb * 128 - c * 512 if c == diag_c else 0
                        ps0 = (c - 2 * ht) * 512 + r
                        ps1 = (c - 2 * ht + 1) * 512
                        nc.tensor.matmul(
                            scA[:, ps0:ps1],
                            lhsA,
                            qt[j][0:64, c * 512 + r:(c + 1) * 512],
                            start=True, stop=True,
                            tile_position=(0, 0),
                        )
                        nc.tensor.matmul(
                            scB[:, ps0:ps1],
                            lhsB,
                            qt[j][64:128, c * 512 + r:(c + 1) * 512],
                            start=True, stop=True,
                            tile_position=(64, 0),
                        )
                    off_in = max(0, sb * 128 - ht * 1024)
                    wv_ = 1024 - off_in
                    off_out = ht * 1024 + off_in - sb * 128
                    nc.scalar.activation(
                        etA[:, off_out:off_out + wv_],
                        scA[:, off_in:1024],
                        AF.Exp,
                        bias=amask_sb[:, sb:sb + 1],
                        scale=0.125,
                    )
                    nc.scalar.activation(
                        etB[:, off_out:off_out + wv_],
                        scB[:, off_in:1024],
                        AF.Exp,
                        bias=amask_sb[:, sb:sb + 1],
                        scale=0.125,
                    )
                nc.gpsimd.tensor_tensor(
                    etA[:, 0:128], etA[:, 0:128], tri_sb[:], mybir.AluOpType.mult
                )
                nc.gpsimd.tensor_tensor(
                    etB[:, 0:128], etB[:, 0:128], tri_sb[:], mybir.AluOpType.mult
                )

            # ---- PV for head h, t-chunk c (rows=65: v out + denom) -----
            def pv(h, c):
                par = h % 2
                s2max = 4 * c + 3
                pL = ps_pv.tile([65, 512], dt.float32, tag="pvL", name="pvL")
                pH = ps_pv.tile([65, 512], dt.float32, tag="pvH", name="pvH")
                for s2 in range(s2max + 1):
                    off_t = max(0, s2 * 128 - c * 512)
                    co = c * 512 + off_t - s2 * 128
                    wv_ = 512 - off_t
                    e = et[par, s2]
                    nc.tensor.matmul(
                        pL[:, off_t:512],
                        v_sb[0:64, s2, h, :],
                        e[0:64, co:co + wv_],
                        start=(s2 == 0), stop=(s2 == s2max),
                        tile_position=(0, 0),
                    )
                    nc.tensor.matmul(
                        pH[:, off_t:512],
                        v_sb[64:128, s2, h, :],
                        e[64:128, co:co + wv_],
                        start=(s2 == 0), stop=(s2 == s2max),
                        tile_position=(64, 0),
                    )
                stage = stpool.tile([65, 512], dt.float32, tag="stage", name="stage")
                nc.vector.tensor_copy(stage[:], pL[:])
                nc.vector.tensor_tensor(
                    stage[:], pH[:], stage[:], mybir.AluOpType.add
                )
                nc.sync.dma_start(out_d[h, c], stage[:])

            # ---- schedule ----------------------------------------------
            # filler queue of (key, fn) in the order pieces become needed
            fillers = []
            for tb in (2, 1, 0):
                fillers.append((("q", 0, tb), lambda tb=tb: proj_qk("q", 0, tb)))
                fillers.append((("k", 0, tb), lambda tb=tb: proj_qk("k", 0, tb)))
            for m in range(16):
                fillers.append((("v", m), lambda m=m: proj_v(m)))
            for j in (1, 2, 3):
                for tb in (3, 2, 1, 0):
                    fillers.append((("q", j, tb), lambda j=j, tb=tb: proj_qk("q", j, tb)))
                    fillers.append((("k", j, tb), lambda j=j, tb=tb: proj_qk("k", j, tb)))
            emitted = set()

            def fill(n=1):
                for _ in range(n):
                    if fillers:
                        key, fn = fillers.pop(0)
                        fn()
                        emitted.add(key)

            def force(*keys):
                # emit queue (in order) until all keys are emitted
                while fillers and not all(k in emitted for k in keys):
                    fill(1)

            # q/k for pair 0, t-block 3 (enough for sb 15..12)
            proj_qk("q", 0, 3)
            proj_qk("k", 0, 3)
            emitted.add(("q", 0, 3))
            emitted.add(("k", 0, 3))

            def do_scores(j, sb):
                # deadline: q/k pieces for the t-blocks this sb touches
                force(("q", j, sb // 4), ("k", j, sb // 4))
                scores(j, sb, gap_fill=fill)

            # pair 0 scores, descending sb
            for sb in range(15, -1, -1):
                do_scores(0, sb)

            # all v pieces precede any PV
            force(("v", 15))

            # pairs: pv(p) c-descending interleaved with scores(p+1)
            # descending-sb.  pv(p, c) must precede scores(p+1, sb) for
            # sb//4 <= c (WAR on et tiles).
            for p in range(4):
                nxt = p + 1 if p < 3 else None
                for c in (3, 2, 1, 0):
                    pv(2 * p, c)
                    fill(1)
                    pv(2 * p + 1, c)
                    if nxt is not None:
                        for sb in range(4 * c + 3, 4 * c - 1, -1):
                            do_scores(nxt, sb)
                    else:
                        fill(2)
            while fillers:
                fill(1)

    nc.compile()
    return nc


def _prep_core_inputs(c, x, mask, Wq, bq, Wk, bk, Wv, bv):
    b, half = divmod(c, 2)
    hs = half * CH

    xT = np.ascontiguousarray(x[b].T.astype(BF16))  # [1024, 2048]
    xTd = np.ascontiguousarray(xT.reshape(8, 128, 4, 512).transpose(2, 1, 0, 3))

    def wT(W):
        Ws = W[hs:hs + CH]  # [512, 1024]
        return np.ascontiguousarray(
            Ws.T.astype(BF16).reshape(8, 128, 512).transpose(1, 0, 2)
        )

    bqk = np.ascontiguousarray(
        np.stack(
            [
                bq[hs:hs + CH].reshape(4, 128).T,
                bk[hs:hs + CH].reshape(4, 128).T,
            ],
            axis=1,
        ).astype(np.float32)
    )  # [128, 2, 4]

    vbias = np.ascontiguousarray(
        np.broadcast_to(
            bv[hs:hs + CH].reshape(8, 64).astype(BF16), (128, 8, 64)
        )
    )

    amask = np.ascontiguousarray(
        mask[b, 0, 0].reshape(16, 128).T.astype(np.float32)
    )
    tri = np.triu(np.ones((128, 128), BF16))

    return {
        "xT": xTd,
        "wqT": wT(Wq),
        "wkT": wT(Wk),
        "wvT": wT(Wv),
        "bqk": bqk,
        "vbias": vbias,
        "amask": amask,
        "tri": tri,
    }


def _run(inputs, trace=False, trace_kwargs=None):
    from concourse.bass_utils import run_bass_kernel_spmd

    args = {k: np.asarray(v) for k, v in inputs.items()}

    # fold LoRA into the q/v weights (mathematically identical)
    Wq = args["Wq"].astype(np.float32) + LORA_SCALE * (
        args["qB"].astype(np.float32) @ args["qA"].astype(np.float32)
    )
    Wv = args["Wv"].astype(np.float32) + LORA_SCALE * (
        args["vB"].astype(np.float32) @ args["vA"].astype(np.float32)
    )

    in_maps = [
        _prep_core_inputs(
            c,
            args["hidden_states"],
            args["attention_mask"],
            Wq, args["bq"], args["Wk"], args["bk"], Wv, args["bv"],
        )
        for c in range(N_CORES)
    ]

    if "nc" not in _cached:
        _cached["nc"] = _build_nc()
    nc = _cached["nc"]

    res = run_bass_kernel_spmd(
        nc, in_maps, core_ids=list(range(N_CORES)), trace=trace,
        **(trace_kwargs or {}),
    )

    full = np.empty((B, T, H), np.float32)
    for c in range(N_CORES):
        b, half = divmod(c, 2)
        hs = half * CH
        o = np.asarray(res.results[c]["out"], np.float32)  # [8, 4, 65, 512]
        for h in range(HPC):
            for ch in range(4):
                blk = o[h, ch]
                full[b, ch * 512:(ch + 1) * 512, hs + h * 64:hs + (h + 1) * 64] = (
                    blk[0:64] / blk[64:65]
                ).T
    return full, res


def kernel(**inputs):
    full, _ = _run(inputs, trace=False)
    return full
